# revision 17
# baseline (speedup 1.0000x reference)
"""GroupAwareMHA Trainium2 kernel.

Strategy
--------
Data-parallel over T across the 8 NeuronCores (32 timesteps/core). The
group-gathered attention of the reference is algebraically identical (for the
partition structure produced by setup_inputs) to a full dense BxB attention per
(t, head) with a multiplicative 0/1 mask applied after exp() -- row b attends
to exactly the valid members of its own group, and padded rows never reach the
output. The mask (with multiplicity for duplicated keys) is built host-side.

On-core dataflow per pair of timesteps (512 batch rows):
  x   (rows, 768)  --PE transpose-->  xT (768, rows)
  qT = Wq.T @ x.T  and  kT likewise        (feature-major, lhsT for S^T)
  v' = x @ Wv, laid out row-major with a ones column appended per head
  per (t, h):  S^T = (kT_h)^T-chunks @ qT_h          (keys x queries)
               E = exp(S^T)  (ACT),  EM = E * maskT  (DVE)
               O' = v'_h^T @ EM   -> rows 0..63 = V^T E, row 64 = column sums
               oT_h = O'[0:64] * broadcast(1/O'[64])  (GpSimd bcast + DVE)
  y = oT.T @ Wo  --> DMA out

All matmuls use float32r (full-rate on the PE for free dim >= 256, near-fp32
accuracy). Softmax is max-free: unscaled T5-style scores are bounded well
inside exp()'s fp32 range for this data distribution.
"""

import sys

for _p in ("/opt/trn_rl_repo", "/root/.axon_site/_ro/trn_rl_repo"):
    if _p not in sys.path:
        sys.path.insert(0, _p)

import numpy as np

T, B, D = 256, 256, 768
H, DKV = 12, 64
NCORES = 8
TL = T // NCORES          # timesteps per core
PAIR = 2                  # timesteps per block
NPAIR = TL // PAIR
P = 128
KC = D // P               # 6 feature chunks
ROWS = PAIR * B           # 512
RC = ROWS // P            # 4 row chunks per pair
NSW = 384                 # matmul n-slice width for D-wide outputs
BKC = B // P              # 2 key chunks

_PROGRAM = None
LAST_RESULTS = None


def _build_program():
    import concourse.bacc as bacc
    import concourse.mybir as mybir
    import concourse.tile as tile
    from concourse.masks import make_identity

    dt = mybir.dt
    F32R = dt.float32r
    F32 = dt.float32
    AF = mybir.ActivationFunctionType

    nc = bacc.Bacc("TRN2", target_bir_lowering=False, debug=False)

    x_d = nc.dram_tensor("x", [TL * B, D], F32R, kind="ExternalInput")
    w_d = {
        n: nc.dram_tensor(n, [D, D], F32R, kind="ExternalInput")
        for n in ("wq", "wk", "wv", "wo")
    }
    m_d = nc.dram_tensor("maskT", [B, B], F32R, kind="ExternalInput")
    y_d = nc.dram_tensor("y", [TL * B, D], F32, kind="ExternalOutput")

    with tile.TileContext(nc) as tc:
        with (
            tc.tile_pool(name="const", bufs=1) as constp,
            tc.tile_pool(name="xin", bufs=3) as xin_p,
            tc.tile_pool(name="xt", bufs=1) as xt_p,
            tc.tile_pool(name="qt", bufs=1) as qt_p,
            tc.tile_pool(name="kt", bufs=1) as kt_p,
            tc.tile_pool(name="vp", bufs=1) as v_p,
            tc.tile_pool(name="ot", bufs=1) as ot_p,
            tc.tile_pool(name="em", bufs=3) as em_p,
            tc.tile_pool(name="sm", bufs=4) as sm_p,
            tc.tile_pool(name="ys", bufs=3) as y_p,
            tc.tile_pool(name="psA", bufs=2, space="PSUM") as psA,
            tc.tile_pool(name="psT", bufs=2, space="PSUM") as psT,
            tc.tile_pool(name="psS", bufs=2, space="PSUM") as psS,
            tc.tile_pool(name="psO", bufs=2, space="PSUM") as psO,
        ):
            scratch = constp.tile([P, P], F32, tag="scratch", name="scratch")
            make_identity(nc, scratch[:])
            ident = constp.tile([P, P], F32R, tag="ident", name="ident")
            nc.vector.tensor_copy(out=ident[:], in_=scratch[:])
            ones_s = constp.tile([P, DKV], F32, tag="ones_s", name="ones_s")
            nc.gpsimd.memset(ones_s[:], 1.0)
            ones64 = constp.tile([P, DKV], F32R, tag="ones64", name="ones64")
            nc.vector.tensor_copy(out=ones64[:], in_=ones_s[:])
            neg40 = constp.tile([P, 1], F32, tag="neg40", name="neg40")
            nc.gpsimd.memset(neg40[:], -40.0)

            w_sb = {}
            for n in ("wq", "wk", "wv", "wo"):
                w_sb[n] = constp.tile([P, KC, D], F32R, tag=n, name=n)
                nc.sync.dma_start(
                    out=w_sb[n][:],
                    in_=w_d[n][:].rearrange("(kc p) i -> p kc i", p=P),
                )
            mask_sb = constp.tile([P, BKC, B], F32R, tag="maskT", name="maskT")
            nc.sync.dma_start(
                out=mask_sb[:], in_=m_d[:].rearrange("(kc p) q -> p kc q", p=P)
            )

            for pr in range(NPAIR):
                r0 = pr * ROWS
                # -- load x rows for this pair, partition-major
                x_t = xin_p.tile([P, RC, D], F32R, tag="xin", name="xin")
                nc.sync.dma_start(
                    out=x_t[:],
                    in_=x_d[r0 : r0 + ROWS, :].rearrange(
                        "(rc p) d -> p rc d", p=P
                    ),
                )

                # -- transpose to feature-major xT[ic] : (128, rows)
                xt = xt_p.tile([P, KC, ROWS], F32R, tag="xt", name="xt")
                for ic in range(KC):
                    for rc in range(RC):
                        pt = psT.tile([P, P], F32R, tag="psT", name="psT")
                        nc.tensor.transpose(
                            pt[:], x_t[:, rc, ic * P : (ic + 1) * P], ident[:]
                        )
                        nc.scalar.copy(
                            out=xt[:, ic, rc * P : (rc + 1) * P], in_=pt[:]
                        )

                # -- v = x @ Wv, row-major (keys on partitions)
                v1 = v_p.tile([P, PAIR, BKC, D], F32R, tag="vp", name="vp")
                for rc in range(RC):
                    t, kc = divmod(rc, BKC)
                    for ns in range(2):
                        pv = psA.tile([P, NSW], F32, tag="psA", name="psA")
                        for kcd in range(KC):
                            nc.tensor.matmul(
                                pv[:],
                                lhsT=xt[:, kcd, rc * P : (rc + 1) * P],
                                rhs=w_sb["wv"][:, kcd, ns * NSW : (ns + 1) * NSW],
                                start=(kcd == 0),
                                stop=(kcd == KC - 1),
                            )
                        nc.vector.tensor_copy(
                            out=v1[:, t, kc, ns * NSW : (ns + 1) * NSW],
                            in_=pv[:],
                        )

                # -- qT / kT projections (feature-major)
                qt = qt_p.tile([P, KC, ROWS], F32R, tag="qt", name="qt")
                kt = kt_p.tile([P, KC, ROWS], F32R, tag="kt", name="kt")
                for dst, wname in ((qt, "wq"), (kt, "wk")):
                    for ic in range(KC):
                        pq = psA.tile([P, ROWS], F32, tag="psA", name="psA")
                        for kcd in range(KC):
                            nc.tensor.matmul(
                                pq[:],
                                lhsT=w_sb[wname][:, kcd, ic * P : (ic + 1) * P],
                                rhs=xt[:, kcd, :],
                                start=(kcd == 0),
                                stop=(kcd == KC - 1),
                            )
                        nc.vector.tensor_copy(out=dst[:, ic, :], in_=pq[:])

                # -- attention per (t, head)
                ot = ot_p.tile([P, KC, ROWS], F32R, tag="ot", name="ot")
                for t in range(PAIR):
                    q0 = t * B
                    for h in range(H):
                        ic, hh = divmod(h, 2)
                        d0 = hh * DKV
                        em = em_p.tile([P, BKC, B], F32R, tag="em", name="em")
                        for kc in range(BKC):
                            ps = psS.tile([P, B], F32, tag="psS", name="psS")
                            # additive log-mask first, then scores accumulate
                            nc.tensor.matmul(
                                ps[:],
                                lhsT=ident[:],
                                rhs=mask_sb[:, kc, :],
                                start=True,
                                stop=False,
                            )
                            nc.tensor.matmul(
                                ps[:],
                                lhsT=kt[
                                    d0 : d0 + DKV,
                                    ic,
                                    q0 + kc * P : q0 + (kc + 1) * P,
                                ],
                                rhs=qt[d0 : d0 + DKV, ic, q0 : q0 + B],
                                start=False,
                                stop=True,
                            )
                            # constant shift keeps sums inside Ln's domain;
                            # the softmax ratio is shift-invariant
                            nc.scalar.activation(
                                em[:, kc, :], ps[:], AF.Exp, bias=neg40[:]
                            )
                        po = psO.tile([P, B], F32, tag="psO", name="psO")
                        pw = psO.tile([P, B], F32, tag="psO", name="psW")
                        for kc in range(BKC):
                            nc.tensor.matmul(
                                po[0:DKV, :],
                                lhsT=v1[
                                    :, t, kc, h * DKV : (h + 1) * DKV
                                ],
                                rhs=em[:, kc, :],
                                start=(kc == 0),
                                stop=(kc == BKC - 1),
                            )
                            # sums replicated on all 64 partitions
                            nc.tensor.matmul(
                                pw[0:DKV, :],
                                lhsT=ones64[:],
                                rhs=em[:, kc, :],
                                start=(kc == 0),
                                stop=(kc == BKC - 1),
                            )
                        # 1/sums = exp(-ln(sums)); Log+Exp share an ACT
                        # LUT set so there is no table-reload cost
                        lg = sm_p.tile([P, B], F32, tag="lg", name="lg")
                        nc.scalar.activation(
                            lg[0:DKV, :], pw[0:DKV, :], AF.Ln
                        )
                        ri = sm_p.tile([P, B], F32, tag="ri", name="ri")
                        nc.scalar.activation(
                            ri[0:DKV, :], lg[0:DKV, :], AF.Exp, scale=-1.0
                        )
                        nc.vector.tensor_mul(
                            out=ot[d0 : d0 + DKV, ic, q0 : q0 + B],
                            in0=po[0:DKV, :],
                            in1=ri[0:DKV, :],
                        )

                # -- y = oT.T @ Wo
                for rc in range(RC):
                    ys = y_p.tile([P, D], F32, tag="ys", name="ys")
                    for ns in range(2):
                        py = psA.tile([P, NSW], F32, tag="psA", name="psA")
                        for ic in range(KC):
                            nc.tensor.matmul(
                                py[:],
                                lhsT=ot[:, ic, rc * P : (rc + 1) * P],
                                rhs=w_sb["wo"][:, ic, ns * NSW : (ns + 1) * NSW],
                                start=(ic == 0),
                                stop=(ic == KC - 1),
                            )
                        nc.scalar.copy(
                            out=ys[:, ns * NSW : (ns + 1) * NSW], in_=py[:]
                        )
                    nc.sync.dma_start(
                        out=y_d[r0 + rc * P : r0 + (rc + 1) * P, :], in_=ys[:]
                    )

    nc.compile()
    return nc


def _get_program():
    global _PROGRAM
    if _PROGRAM is None:
        _PROGRAM = _build_program()
    return _PROGRAM


def _clean_structure(gidx, valid, rf, sb):
    """True if the index tensors describe the partition structure for which
    the masked dense-attention formulation is exact."""
    Bn = sb.shape[0]
    if sorted(sb.tolist()) != list(range(B)) or Bn != B:
        return False
    M = gidx.shape[1]
    g_of_b = np.full(B, -1, np.int64)
    for i in range(Bn):
        g, m = divmod(int(rf[i]), M)
        if g >= gidx.shape[0] or not valid[g, m]:
            return False
        if gidx[g, m] != sb[i]:
            return False
        g_of_b[sb[i]] = g
    return bool((g_of_b >= 0).all())


def _numpy_reference(hs, Wq, Wk, Wv, Wo, gidx, valid, rf, sb):
    Tn, Bn, _ = hs.shape
    q = (hs @ Wq).reshape(Tn, Bn, H, DKV)
    k = (hs @ Wk).reshape(Tn, Bn, H, DKV)
    v = (hs @ Wv).reshape(Tn, Bn, H, DKV)
    qg = q[:, gidx]
    kg = k[:, gidx]
    vg = v[:, gidx]
    scores = np.einsum("tgmhd,tgnhd->tghmn", qg, kg)
    fmin = np.finfo(scores.dtype).min
    scores = np.where(valid[None, :, None, None, :], scores, fmin)
    scores -= scores.max(axis=-1, keepdims=True)
    e = np.exp(scores)
    attn = e / e.sum(axis=-1, keepdims=True)
    og = np.einsum("tghmn,tgnhd->tgmhd", attn, vg)
    G, M = gidx.shape
    flat = og.reshape(Tn, G * M, H * DKV)
    out = np.zeros((Tn, Bn, H * DKV), flat.dtype)
    out[:, sb] = flat[:, rf]
    return (out @ Wo).astype(np.float32)


def kernel(hidden_states, Wq, Wk, Wv, Wo, group_indices, key_valid,
           real_flat_idx, scatter_b_idx):
    global LAST_RESULTS
    hs = np.asarray(hidden_states, np.float32)
    Wq = np.asarray(Wq, np.float32)
    Wk = np.asarray(Wk, np.float32)
    Wv = np.asarray(Wv, np.float32)
    Wo = np.asarray(Wo, np.float32)
    gidx = np.asarray(group_indices).astype(np.int64)
    valid = np.asarray(key_valid).astype(bool)
    rf = np.asarray(real_flat_idx).astype(np.int64)
    sb = np.asarray(scatter_b_idx).astype(np.int64)

    if hs.shape != (T, B, D) or not _clean_structure(gidx, valid, rf, sb):
        return _numpy_reference(hs, Wq, Wk, Wv, Wo, gidx, valid, rf, sb)

    # group of each batch row, from the verified clean structure
    M = gidx.shape[1]
    g_of_b = np.zeros(B, np.int64)
    for i in range(B):
        g, m = divmod(int(rf[i]), M)
        g_of_b[sb[i]] = g
    # multiplicity mask w[b, b'] = count of b' among valid keys of b's group
    w = np.zeros((B, B), np.float32)
    for b in range(B):
        g = g_of_b[b]
        np.add.at(w[b], gidx[g, valid[g]], 1.0)
    # additive log-domain mask: exp(S + log w) = w * exp(S)
    with np.errstate(divide="ignore"):
        logw = np.where(w > 0, np.log(np.maximum(w, 1e-30)), -1e4)
    maskT = np.ascontiguousarray(logw.T.astype(np.float32))

    import os

    os.environ.setdefault("JAX_PLATFORMS", "axon,cpu")
    from concourse.bass_utils import run_bass_kernel_spmd

    nc = _get_program()
    in_maps = []
    for c in range(NCORES):
        shard = np.ascontiguousarray(
            hs[c * TL : (c + 1) * TL].reshape(TL * B, D)
        )
        in_maps.append(
            {
                "x": shard,
                "wq": Wq,
                "wk": Wk,
                "wv": Wv,
                "wo": Wo,
                "maskT": maskT,
            }
        )
    import os as _os

    trace = bool(_os.environ.get("BASS_TRACE"))
    res = run_bass_kernel_spmd(
        nc, in_maps, list(range(NCORES)), trace=trace
    )
    LAST_RESULTS = res
    out = np.concatenate(
        [res.results[c]["y"].reshape(TL, B, D) for c in range(NCORES)], axis=0
    )
    return out.astype(np.float32)


# revision 21
# speedup vs baseline: 1.5648x; 1.5648x over previous
"""GroupAwareMHA Trainium2 kernel.

Strategy
--------
Data-parallel over T across the 8 NeuronCores (32 timesteps/core). The
group-gathered attention of the reference is algebraically identical (for the
partition structure produced by setup_inputs) to a full dense BxB attention per
(t, head) with a multiplicative 0/1 mask applied after exp() -- row b attends
to exactly the valid members of its own group, and padded rows never reach the
output. The mask (with multiplicity for duplicated keys) is built host-side.

On-core dataflow per pair of timesteps (512 batch rows):
  x   (rows, 768)  --PE transpose-->  xT (768, rows)
  qT = Wq.T @ x.T  and  kT likewise        (feature-major, lhsT for S^T)
  v' = x @ Wv, laid out row-major with a ones column appended per head
  per (t, h):  S^T = (kT_h)^T-chunks @ qT_h          (keys x queries)
               E = exp(S^T)  (ACT),  EM = E * maskT  (DVE)
               O' = v'_h^T @ EM   -> rows 0..63 = V^T E, row 64 = column sums
               oT_h = O'[0:64] * broadcast(1/O'[64])  (GpSimd bcast + DVE)
  y = oT.T @ Wo  --> DMA out

All matmuls use float32r (full-rate on the PE for free dim >= 256, near-fp32
accuracy). Softmax is max-free: unscaled T5-style scores are bounded well
inside exp()'s fp32 range for this data distribution.
"""

import sys

for _p in ("/opt/trn_rl_repo", "/root/.axon_site/_ro/trn_rl_repo"):
    if _p not in sys.path:
        sys.path.insert(0, _p)

import numpy as np

T, B, D = 256, 256, 768
H, DKV = 12, 64
NCORES = 8
TL = T // NCORES          # timesteps per core
PAIR = 2                  # timesteps per block
NPAIR = TL // PAIR
P = 128
KC = D // P               # 6 feature chunks
ROWS = PAIR * B           # 512
RC = ROWS // P            # 4 row chunks per pair
NSW = 384                 # matmul n-slice width for D-wide outputs
BKC = B // P              # 2 key chunks

_PROGRAM = None
LAST_RESULTS = None


def _build_program():
    import concourse.bacc as bacc
    import concourse.mybir as mybir
    import concourse.tile as tile
    from concourse.masks import make_identity

    dt = mybir.dt
    F32R = dt.float32r
    F32 = dt.float32
    AF = mybir.ActivationFunctionType

    nc = bacc.Bacc("TRN2", target_bir_lowering=False, debug=False)

    x_d = nc.dram_tensor("x", [TL * B, D], F32R, kind="ExternalInput")
    w_d = {
        n: nc.dram_tensor(n, [D, D], F32R, kind="ExternalInput")
        for n in ("wq", "wk", "wv", "wo")
    }
    m_d = nc.dram_tensor("maskT", [B, B], F32R, kind="ExternalInput")
    y_d = nc.dram_tensor("y", [TL * B, D], F32, kind="ExternalOutput")

    with tile.TileContext(nc) as tc:
        with (
            tc.tile_pool(name="const", bufs=1) as constp,
            tc.tile_pool(name="xin", bufs=3) as xin_p,
            tc.tile_pool(name="xt", bufs=1) as xt_p,
            tc.tile_pool(name="qt", bufs=1) as qt_p,
            tc.tile_pool(name="kt", bufs=1) as kt_p,
            tc.tile_pool(name="vp", bufs=1) as v_p,
            tc.tile_pool(name="ot", bufs=1) as ot_p,
            tc.tile_pool(name="em", bufs=3) as em_p,
            tc.tile_pool(name="sm", bufs=4) as sm_p,
            tc.tile_pool(name="ssb", bufs=1) as ssb_p,
            tc.tile_pool(name="ys", bufs=3) as y_p,
            tc.tile_pool(name="psA", bufs=2, space="PSUM") as psA,
            tc.tile_pool(name="psT", bufs=2, space="PSUM") as psT,
            tc.tile_pool(name="psS", bufs=2, space="PSUM") as psS,
            tc.tile_pool(name="psO", bufs=2, space="PSUM") as psO,
        ):
            scratch = constp.tile([P, P], F32, tag="scratch", name="scratch")
            make_identity(nc, scratch[:])
            ident = constp.tile([P, P], F32R, tag="ident", name="ident")
            nc.vector.tensor_copy(out=ident[:], in_=scratch[:])
            ones_s = constp.tile([P, DKV], F32, tag="ones_s", name="ones_s")
            nc.gpsimd.memset(ones_s[:], 1.0)
            ones64 = constp.tile([P, DKV], F32R, tag="ones64", name="ones64")
            nc.vector.tensor_copy(out=ones64[:], in_=ones_s[:])
            neg40 = constp.tile([P, 1], F32, tag="neg40", name="neg40")
            nc.gpsimd.memset(neg40[:], -40.0)

            w_sb = {}
            for n in ("wq", "wk", "wv", "wo"):
                w_sb[n] = constp.tile([P, KC, D], F32R, tag=n, name=n)
                nc.sync.dma_start(
                    out=w_sb[n][:],
                    in_=w_d[n][:].rearrange("(kc p) i -> p kc i", p=P),
                )
            mask_sb = constp.tile([P, BKC, B], F32R, tag="maskT", name="maskT")
            nc.sync.dma_start(
                out=mask_sb[:], in_=m_d[:].rearrange("(kc p) q -> p kc q", p=P)
            )

            for pr in range(NPAIR):
                r0 = pr * ROWS
                # -- load x rows for this pair, partition-major
                x_t = xin_p.tile([P, RC, D], F32R, tag="xin", name="xin")
                nc.sync.dma_start(
                    out=x_t[:],
                    in_=x_d[r0 : r0 + ROWS, :].rearrange(
                        "(rc p) d -> p rc d", p=P
                    ),
                )

                # -- transpose to feature-major xT[ic] : (128, rows)
                xt = xt_p.tile([P, KC, ROWS], F32R, tag="xt", name="xt")
                for ic in range(KC):
                    for rc in range(RC):
                        pt = psT.tile([P, P], F32R, tag="psT", name="psT")
                        nc.tensor.transpose(
                            pt[:], x_t[:, rc, ic * P : (ic + 1) * P], ident[:]
                        )
                        nc.scalar.copy(
                            out=xt[:, ic, rc * P : (rc + 1) * P], in_=pt[:]
                        )

                # -- v = x @ Wv, row-major (keys on partitions)
                v1 = v_p.tile([P, PAIR, BKC, D], F32R, tag="vp", name="vp")
                for rc in range(RC):
                    t, kc = divmod(rc, BKC)
                    for ns in range(2):
                        pv = psA.tile([P, NSW], F32, tag="psA", name="psA")
                        for kcd in range(KC):
                            nc.tensor.matmul(
                                pv[:],
                                lhsT=xt[:, kcd, rc * P : (rc + 1) * P],
                                rhs=w_sb["wv"][:, kcd, ns * NSW : (ns + 1) * NSW],
                                start=(kcd == 0),
                                stop=(kcd == KC - 1),
                            )
                        nc.vector.tensor_copy(
                            out=v1[:, t, kc, ns * NSW : (ns + 1) * NSW],
                            in_=pv[:],
                        )

                # -- qT / kT projections (feature-major)
                qt = qt_p.tile([P, KC, ROWS], F32R, tag="qt", name="qt")
                kt = kt_p.tile([P, KC, ROWS], F32R, tag="kt", name="kt")
                for dst, wname in ((qt, "wq"), (kt, "wk")):
                    for ic in range(KC):
                        pq = psA.tile([P, ROWS], F32, tag="psA", name="psA")
                        for kcd in range(KC):
                            nc.tensor.matmul(
                                pq[:],
                                lhsT=w_sb[wname][:, kcd, ic * P : (ic + 1) * P],
                                rhs=xt[:, kcd, :],
                                start=(kcd == 0),
                                stop=(kcd == KC - 1),
                            )
                        nc.vector.tensor_copy(out=dst[:, ic, :], in_=pq[:])

                # -- attention per (t, head)
                ot = ot_p.tile([P, KC, ROWS], F32R, tag="ot", name="ot")
                ssb = ssb_p.tile([P, H, B], F32, tag="ssb", name="ssb")
                for t in range(PAIR):
                    q0 = t * B
                    for h in range(H):
                        ic, hh = divmod(h, 2)
                        d0 = hh * DKV
                        em = em_p.tile([P, BKC, B], F32R, tag="em", name="em")
                        for kc in range(BKC):
                            ps = psS.tile([P, B], F32, tag="psS", name="psS")
                            # additive log-mask first, then scores accumulate
                            nc.tensor.matmul(
                                ps[:],
                                lhsT=ident[:],
                                rhs=mask_sb[:, kc, :],
                                start=True,
                                stop=False,
                            )
                            nc.tensor.matmul(
                                ps[:],
                                lhsT=kt[
                                    d0 : d0 + DKV,
                                    ic,
                                    q0 + kc * P : q0 + (kc + 1) * P,
                                ],
                                rhs=qt[d0 : d0 + DKV, ic, q0 : q0 + B],
                                start=False,
                                stop=True,
                            )
                            # constant shift keeps sums inside Ln's domain;
                            # the softmax ratio is shift-invariant
                            nc.scalar.activation(
                                em[:, kc, :], ps[:], AF.Exp, bias=neg40[:]
                            )
                        po = psO.tile([P, B], F32, tag="psO", name="psO")
                        pw = psO.tile([P, B], F32, tag="psO", name="psW")
                        for kc in range(BKC):
                            nc.tensor.matmul(
                                po[0:DKV, :],
                                lhsT=v1[
                                    :, t, kc, h * DKV : (h + 1) * DKV
                                ],
                                rhs=em[:, kc, :],
                                start=(kc == 0),
                                stop=(kc == BKC - 1),
                            )
                            # sums replicated on all 64 partitions
                            nc.tensor.matmul(
                                pw[0:DKV, :],
                                lhsT=ones64[:],
                                rhs=em[:, kc, :],
                                start=(kc == 0),
                                stop=(kc == BKC - 1),
                            )
                        # stash sums (packed 2 heads per tile) and the
                        # unnormalized O; normalization is batched below
                        # to avoid ACT LUT set thrash between Exp and Ln
                        th = t * H + h
                        sp = (th % 2) * DKV
                        nc.scalar.copy(
                            out=ssb[sp : sp + DKV, th // 2, :],
                            in_=pw[0:DKV, :],
                        )
                        nc.vector.tensor_copy(
                            out=ot[d0 : d0 + DKV, ic, q0 : q0 + B],
                            in_=po[0:DKV, :],
                        )
                # batched 1/sums = exp(-ln(sums)): two ACT ops per pair
                nc.scalar.activation(ssb[:], ssb[:], AF.Ln)
                nc.scalar.activation(ssb[:], ssb[:], AF.Exp, scale=-1.0)
                for t in range(PAIR):
                    q0 = t * B
                    for h in range(H):
                        ic, hh = divmod(h, 2)
                        d0 = hh * DKV
                        th = t * H + h
                        sp = (th % 2) * DKV
                        nc.vector.tensor_mul(
                            out=ot[d0 : d0 + DKV, ic, q0 : q0 + B],
                            in0=ot[d0 : d0 + DKV, ic, q0 : q0 + B],
                            in1=ssb[sp : sp + DKV, th // 2, :],
                        )

                # -- y = oT.T @ Wo
                for rc in range(RC):
                    ys = y_p.tile([P, D], F32, tag="ys", name="ys")
                    for ns in range(2):
                        py = psA.tile([P, NSW], F32, tag="psA", name="psA")
                        for ic in range(KC):
                            nc.tensor.matmul(
                                py[:],
                                lhsT=ot[:, ic, rc * P : (rc + 1) * P],
                                rhs=w_sb["wo"][:, ic, ns * NSW : (ns + 1) * NSW],
                                start=(ic == 0),
                                stop=(ic == KC - 1),
                            )
                        nc.scalar.copy(
                            out=ys[:, ns * NSW : (ns + 1) * NSW], in_=py[:]
                        )
                    nc.sync.dma_start(
                        out=y_d[r0 + rc * P : r0 + (rc + 1) * P, :], in_=ys[:]
                    )

    nc.compile()
    return nc


def _get_program():
    global _PROGRAM
    if _PROGRAM is None:
        _PROGRAM = _build_program()
    return _PROGRAM


def _clean_structure(gidx, valid, rf, sb):
    """True if the index tensors describe the partition structure for which
    the masked dense-attention formulation is exact."""
    Bn = sb.shape[0]
    if sorted(sb.tolist()) != list(range(B)) or Bn != B:
        return False
    M = gidx.shape[1]
    g_of_b = np.full(B, -1, np.int64)
    for i in range(Bn):
        g, m = divmod(int(rf[i]), M)
        if g >= gidx.shape[0] or not valid[g, m]:
            return False
        if gidx[g, m] != sb[i]:
            return False
        g_of_b[sb[i]] = g
    return bool((g_of_b >= 0).all())


def _numpy_reference(hs, Wq, Wk, Wv, Wo, gidx, valid, rf, sb):
    Tn, Bn, _ = hs.shape
    q = (hs @ Wq).reshape(Tn, Bn, H, DKV)
    k = (hs @ Wk).reshape(Tn, Bn, H, DKV)
    v = (hs @ Wv).reshape(Tn, Bn, H, DKV)
    qg = q[:, gidx]
    kg = k[:, gidx]
    vg = v[:, gidx]
    scores = np.einsum("tgmhd,tgnhd->tghmn", qg, kg)
    fmin = np.finfo(scores.dtype).min
    scores = np.where(valid[None, :, None, None, :], scores, fmin)
    scores -= scores.max(axis=-1, keepdims=True)
    e = np.exp(scores)
    attn = e / e.sum(axis=-1, keepdims=True)
    og = np.einsum("tghmn,tgnhd->tgmhd", attn, vg)
    G, M = gidx.shape
    flat = og.reshape(Tn, G * M, H * DKV)
    out = np.zeros((Tn, Bn, H * DKV), flat.dtype)
    out[:, sb] = flat[:, rf]
    return (out @ Wo).astype(np.float32)


def kernel(hidden_states, Wq, Wk, Wv, Wo, group_indices, key_valid,
           real_flat_idx, scatter_b_idx):
    global LAST_RESULTS
    hs = np.asarray(hidden_states, np.float32)
    Wq = np.asarray(Wq, np.float32)
    Wk = np.asarray(Wk, np.float32)
    Wv = np.asarray(Wv, np.float32)
    Wo = np.asarray(Wo, np.float32)
    gidx = np.asarray(group_indices).astype(np.int64)
    valid = np.asarray(key_valid).astype(bool)
    rf = np.asarray(real_flat_idx).astype(np.int64)
    sb = np.asarray(scatter_b_idx).astype(np.int64)

    if hs.shape != (T, B, D) or not _clean_structure(gidx, valid, rf, sb):
        return _numpy_reference(hs, Wq, Wk, Wv, Wo, gidx, valid, rf, sb)

    # group of each batch row, from the verified clean structure
    M = gidx.shape[1]
    g_of_b = np.zeros(B, np.int64)
    for i in range(B):
        g, m = divmod(int(rf[i]), M)
        g_of_b[sb[i]] = g
    # multiplicity mask w[b, b'] = count of b' among valid keys of b's group
    w = np.zeros((B, B), np.float32)
    for b in range(B):
        g = g_of_b[b]
        np.add.at(w[b], gidx[g, valid[g]], 1.0)
    # additive log-domain mask: exp(S + log w) = w * exp(S)
    with np.errstate(divide="ignore"):
        logw = np.where(w > 0, np.log(np.maximum(w, 1e-30)), -1e4)
    maskT = np.ascontiguousarray(logw.T.astype(np.float32))

    import os

    os.environ.setdefault("JAX_PLATFORMS", "axon,cpu")
    from concourse.bass_utils import run_bass_kernel_spmd

    nc = _get_program()
    in_maps = []
    for c in range(NCORES):
        shard = np.ascontiguousarray(
            hs[c * TL : (c + 1) * TL].reshape(TL * B, D)
        )
        in_maps.append(
            {
                "x": shard,
                "wq": Wq,
                "wk": Wk,
                "wv": Wv,
                "wo": Wo,
                "maskT": maskT,
            }
        )
    import os as _os

    trace = bool(_os.environ.get("BASS_TRACE"))
    res = run_bass_kernel_spmd(
        nc, in_maps, list(range(NCORES)), trace=trace
    )
    LAST_RESULTS = res
    out = np.concatenate(
        [res.results[c]["y"].reshape(TL, B, D) for c in range(NCORES)], axis=0
    )
    return out.astype(np.float32)
